# revision 1
# baseline (speedup 1.0000x reference)
"""ErnieLayout self-attention Trainium2 kernel.

Shards batch x heads over 8 NeuronCores: cores 0-3 take batch 0, cores 4-7
take batch 1, 3 heads each (data-parallel on batch, tensor-parallel on heads).
No cross-core communication; host gathers per-core [S, 192] outputs.

Per-core pipeline (all matmuls bf16 with fp32 PSUM accumulation):
  - one-time: hs -> hsT, W -> W^T via PE transpose; projections produce
    qT/kT [64, S] (scale + bias folded) and v_aug [S, 65] (ones col -> sums)
  - per (s-tile, head): QK^T on PE; GPSIMD adds rel+rel2d; DVE fuses
    mask*(-30000) + that, then adds PSUM scores -> bf16 logits; PE transposes
    logit blocks; ACT exps PSUM->SBUF (fused eviction) -> probsT; PV matmul
    gives ctx + row-sums; DVE reciprocal * ctx; DMA out.
"""

import numpy as np

B, S, HID = 2, 2048, 768
NH, HD = 12, 64
NCORES = 8
HPC = 3          # heads per core
NEG = -30000.0   # additive mask constant; exp(-30000) == 0.0 in fp32

_CACHE = {}


def _build():
    from contextlib import ExitStack

    import concourse.bacc as bacc
    import concourse.tile as tile
    from concourse import mybir
    from concourse.masks import make_identity

    fp32 = mybir.dt.float32
    bf16 = mybir.dt.bfloat16
    i32 = mybir.dt.int32
    Alu = mybir.AluOpType

    nc = bacc.Bacc(
        "TRN2",
        target_bir_lowering=False,
        debug=False,
        enable_asserts=False,
        num_devices=NCORES,
    )

    hs_d = nc.dram_tensor("hs", (S, HID), fp32, kind="ExternalInput").ap()
    wq_d = nc.dram_tensor("wq", (HPC * HD, HID), fp32, kind="ExternalInput").ap()
    wk_d = nc.dram_tensor("wk", (HPC * HD, HID), fp32, kind="ExternalInput").ap()
    wv_d = nc.dram_tensor("wv", (HPC * HD, HID), fp32, kind="ExternalInput").ap()
    bq_d = nc.dram_tensor("bq", (1, HPC * HD), fp32, kind="ExternalInput").ap()
    bk_d = nc.dram_tensor("bk", (1, HPC * HD), fp32, kind="ExternalInput").ap()
    bv_d = nc.dram_tensor("bv", (1, HPC * HD), fp32, kind="ExternalInput").ap()
    rel_d = nc.dram_tensor("rel", (HPC, S, S), fp32, kind="ExternalInput").ap()
    rel2_d = nc.dram_tensor("rel2", (HPC, S, S), fp32, kind="ExternalInput").ap()
    mask_d = nc.dram_tensor("mask", (S, S), i32, kind="ExternalInput").ap()
    out_d = nc.dram_tensor("out", (S, HPC * HD), fp32, kind="ExternalOutput").ap()

    NWID = HPC * HD    # 192
    NSC = S // 128     # 16 s-tiles
    NKC = HID // 128   # 6 contraction chunks

    with tile.TileContext(nc) as tc, ExitStack() as top:
        persist = top.enter_context(tc.tile_pool(name="persist", bufs=1))

        ident = persist.tile([128, 128], bf16, tag="ident")
        make_identity(nc, ident)
        ones_row = persist.tile([1, S], bf16, tag="ones_row")
        nc.vector.memset(ones_row, 1.0)

        # hsT[:, kc, s] = hs[s, kc*128 + p] as bf16
        hsT = persist.tile([128, NKC, S], bf16, tag="hsT")
        # w?T[:, kc, i] = W[i, kc*128 + p] as bf16
        wT = [
            persist.tile([128, NKC, NWID], bf16, tag=f"wT{w}", name=f"wT{w}")
            for w in range(3)
        ]
        bias_sb = [
            persist.tile([1, NWID], bf16, tag=f"bias{w}", name=f"bias{w}")
            for w in range(3)
        ]

        # ---- Phase 0: load + transpose weights and hidden states ----
        with ExitStack() as ph:
            stage = ph.enter_context(tc.tile_pool(name="stage", bufs=3))
            tps = ph.enter_context(tc.tile_pool(name="tps", bufs=3, space="PSUM"))

            for w, (w_d, b_d) in enumerate(((wq_d, bq_d), (wk_d, bk_d), (wv_d, bv_d))):
                btmp = stage.tile([1, NWID], fp32, tag="btmp")
                nc.sync.dma_start(out=btmp, in_=b_d)
                nc.vector.tensor_copy(bias_sb[w], btmp)
                for rc in range(2):
                    wrow = stage.tile([96, HID], fp32, tag="wrow")
                    nc.sync.dma_start(out=wrow, in_=w_d[rc * 96 : (rc + 1) * 96, :])
                    wrow_bf = stage.tile([96, HID], bf16, tag="wrow_bf")
                    nc.vector.tensor_copy(wrow_bf, wrow)
                    for kc in range(NKC):
                        tp = tps.tile([128, 96], bf16, tag="tpw")
                        nc.tensor.transpose(
                            tp, wrow_bf[:, kc * 128 : (kc + 1) * 128], ident[:96, :96]
                        )
                        nc.scalar.copy(wT[w][:, kc, rc * 96 : (rc + 1) * 96], tp)

            for sc in range(NSC):
                hrow = stage.tile([128, HID], fp32, tag="hrow")
                nc.sync.dma_start(out=hrow, in_=hs_d[sc * 128 : (sc + 1) * 128, :])
                hrow_bf = stage.tile([128, HID], bf16, tag="hrow_bf")
                nc.vector.tensor_copy(hrow_bf, hrow)
                for kc in range(NKC):
                    tp2 = tps.tile([128, 128], bf16, tag="tph")
                    nc.tensor.transpose(tp2, hrow_bf[:, kc * 128 : (kc + 1) * 128], ident)
                    nc.scalar.copy(hsT[:, kc, sc * 128 : (sc + 1) * 128], tp2)

        # ---- Phase 1: projections ----
        # qT/kT: [64, S] per head (already scaled+biased); v_aug: [128, sc, 65]
        qT = [persist.tile([64, S], bf16, tag=f"qT{h}", name=f"qT{h}") for h in range(HPC)]
        kT = [persist.tile([64, S], bf16, tag=f"kT{h}", name=f"kT{h}") for h in range(HPC)]
        v_aug = [
            persist.tile([128, NSC, HD + 1], bf16, tag=f"vaug{h}", name=f"vaug{h}")
            for h in range(HPC)
        ]
        for h in range(HPC):
            nc.vector.memset(v_aug[h], 1.0)

        with ExitStack() as ph:
            pps = ph.enter_context(tc.tile_pool(name="pps", bufs=3, space="PSUM"))

            for h in range(HPC):
                for w, dst in ((0, qT[h]), (1, kT[h])):
                    for nch in range(S // 512):
                        ps = pps.tile([64, 512], fp32, tag="ps_qk")
                        sl = slice(nch * 512, (nch + 1) * 512)
                        for kc in range(NKC):
                            nc.tensor.matmul(
                                ps,
                                lhsT=wT[w][:, kc, h * HD : (h + 1) * HD],
                                rhs=hsT[:, kc, sl],
                                start=(kc == 0),
                                stop=False,
                            )
                        nc.tensor.matmul(
                            ps,
                            lhsT=bias_sb[w][:, h * HD : (h + 1) * HD],
                            rhs=ones_row[:, sl],
                            start=False,
                            stop=True,
                        )
                        nc.scalar.copy(dst[:, sl], ps)

            for sc in range(NSC):
                psv = pps.tile([128, NWID], fp32, tag="ps_v")
                ssl = slice(sc * 128, (sc + 1) * 128)
                for kc in range(NKC):
                    nc.tensor.matmul(
                        psv,
                        lhsT=hsT[:, kc, ssl],
                        rhs=wT[2][:, kc, :],
                        start=(kc == 0),
                        stop=False,
                    )
                nc.tensor.matmul(
                    psv, lhsT=ones_row[:, ssl], rhs=bias_sb[2], start=False, stop=True
                )
                for h in range(HPC):
                    nc.scalar.copy(
                        v_aug[h][:, sc, 0:HD], psv[:, h * HD : (h + 1) * HD]
                    )

        # ---- Phase 2: attention ----
        with ExitStack() as ph:
            mp = ph.enter_context(tc.tile_pool(name="mp", bufs=2))
            rp = ph.enter_context(tc.tile_pool(name="rp", bufs=3))
            cp = ph.enter_context(tc.tile_pool(name="cp", bufs=2))
            lp = ph.enter_context(tc.tile_pool(name="lp", bufs=2))
            prp = ph.enter_context(tc.tile_pool(name="prp", bufs=2))
            op = ph.enter_context(tc.tile_pool(name="op", bufs=3))
            sps = ph.enter_context(tc.tile_pool(name="sps", bufs=2, space="PSUM"))
            tps2 = ph.enter_context(tc.tile_pool(name="tps2", bufs=2, space="PSUM"))
            cps = ph.enter_context(tc.tile_pool(name="cps", bufs=2, space="PSUM"))

            for si in range(NSC):
                ssl = slice(si * 128, (si + 1) * 128)
                mask_t = mp.tile([128, S], i32, tag="mask")
                nc.sync.dma_start(out=mask_t, in_=mask_d[ssl, :])
                for h in range(HPC):
                    rel_t = rp.tile([128, S], fp32, tag="rel")
                    nc.sync.dma_start(out=rel_t, in_=rel_d[h, ssl, :])
                    rel2_t = rp.tile([128, S], fp32, tag="rel2")
                    nc.sync.dma_start(out=rel2_t, in_=rel2_d[h, ssl, :])

                    # c1 = rel + rel2d       (GPSIMD, frees DVE)
                    c1 = cp.tile([128, S], fp32, tag="c1")
                    nc.gpsimd.tensor_add(c1, rel_t, rel2_t)
                    # c2 = mask * NEG + c1   (DVE fused scalar_tensor_tensor)
                    c2 = cp.tile([128, S], fp32, tag="c2")
                    nc.vector.scalar_tensor_tensor(
                        out=c2,
                        in0=mask_t,
                        scalar=NEG,
                        in1=c1,
                        op0=Alu.mult,
                        op1=Alu.add,
                    )

                    # scores chunks + bias add -> bf16 logits
                    logits = lp.tile([128, S], bf16, tag="logits")
                    for tch in range(S // 512):
                        tsl = slice(tch * 512, (tch + 1) * 512)
                        sc_ps = sps.tile([128, 512], fp32, tag="sc")
                        nc.tensor.matmul(
                            sc_ps, lhsT=qT[h][:, ssl], rhs=kT[h][:, tsl],
                            start=True, stop=True,
                        )
                        nc.vector.tensor_add(logits[:, tsl], sc_ps, c2[:, tsl])

                    # transpose logit blocks: ltp[:, tb*128+j] rows=t, cols=s
                    ltp = tps2.tile([128, S], bf16, tag="ltp")
                    for tb in range(NSC):
                        bsl = slice(tb * 128, (tb + 1) * 128)
                        nc.tensor.transpose(ltp[:, bsl], logits[:, bsl], ident)

                    probsT = prp.tile([128, S], bf16, tag="probsT")
                    nc.scalar.activation(
                        probsT, ltp, mybir.ActivationFunctionType.Exp
                    )

                    ctx_ps = cps.tile([128, HD + 1], fp32, tag="ctx")
                    for tb in range(NSC):
                        bsl = slice(tb * 128, (tb + 1) * 128)
                        nc.tensor.matmul(
                            ctx_ps,
                            lhsT=probsT[:, bsl],
                            rhs=v_aug[h][:, tb, :],
                            start=(tb == 0),
                            stop=(tb == NSC - 1),
                        )

                    rec = op.tile([128, 1], fp32, tag="rec")
                    nc.vector.reciprocal(rec, ctx_ps[:, HD : HD + 1])
                    o_t = op.tile([128, HD], fp32, tag="o_t")
                    nc.vector.tensor_scalar(
                        out=o_t, in0=ctx_ps[:, 0:HD], scalar1=rec, scalar2=None,
                        op0=Alu.mult,
                    )
                    nc.sync.dma_start(
                        out=out_d[ssl, h * HD : (h + 1) * HD], in_=o_t
                    )

    nc.compile()
    return nc


def get_nc():
    if "nc" not in _CACHE:
        _CACHE["nc"] = _build()
    return _CACHE["nc"]


def make_in_maps(
    hidden_states, rel_pos, rel_2d_pos, attention_mask, Wq, bq, Wk, bk, Wv, bv
):
    hidden_states = np.asarray(hidden_states, dtype=np.float32)
    rel_pos = np.asarray(rel_pos, dtype=np.float32)
    rel_2d_pos = np.asarray(rel_2d_pos, dtype=np.float32)
    attention_mask = np.asarray(attention_mask, dtype=np.int32)
    Wq = np.asarray(Wq, dtype=np.float32)
    bq = np.asarray(bq, dtype=np.float32)
    Wk = np.asarray(Wk, dtype=np.float32)
    bk = np.asarray(bk, dtype=np.float32)
    Wv = np.asarray(Wv, dtype=np.float32)
    bv = np.asarray(bv, dtype=np.float32)

    scale = 1.0 / np.sqrt(np.float32(HD))
    in_maps = []
    for c in range(NCORES):
        b = c // 4
        h0 = HPC * (c % 4)
        rsl = slice(HD * h0, HD * (h0 + HPC))
        in_maps.append(
            {
                "hs": hidden_states[b],
                "wq": Wq[rsl] * scale,
                "wk": Wk[rsl],
                "wv": Wv[rsl],
                "bq": (bq[rsl] * scale).reshape(1, -1),
                "bk": bk[rsl].reshape(1, -1),
                "bv": bv[rsl].reshape(1, -1),
                "rel": rel_pos[b, h0 : h0 + HPC],
                "rel2": rel_2d_pos[b, h0 : h0 + HPC],
                "mask": attention_mask[b, 0],
            }
        )
    return in_maps


def gather_out(results):
    out = np.empty((B, S, HID), dtype=np.float32)
    for c in range(NCORES):
        b = c // 4
        g = c % 4
        out[b, :, g * HPC * HD : (g + 1) * HPC * HD] = results[c]["out"]
    return out


def kernel(**inputs) -> np.ndarray:
    from concourse import bass_utils

    nc = get_nc()
    in_maps = make_in_maps(**inputs)
    res = bass_utils.run_bass_kernel_spmd(nc, in_maps, core_ids=list(range(NCORES)))
    return gather_out(res.results)
